# revision 82
# baseline (speedup 1.0000x reference)
"""TRN2 Bass kernel for nn_NaiveReweightedLoss (reweighted per-class BCE-style loss).

Reference semantics (N=32768 samples, C=1000 classes, t in {0,1}):
    B_c = sum_i t * softplus(-p),  C_c = sum_i (1-t) * softplus(p)
    n_pos_c = sum_i t, n_neg_c = N - n_pos_c
    valid = (n_pos>0)&(n_neg>0)
    loss = mean over valid classes of B/max(n_pos,1) + C/max(n_neg,1)

Device algorithm (data-parallel over rows, 8 cores x 4096 rows):
  Even/odd split of softplus kills one of the two ACT passes the exact
  exp+ln route needs:  softplus(z) = z/2 + E(m),  m = -|z|,
  E(m) = ln(2 cosh(m/2)) is EVEN, approximated by one tabled function:
      E(m) ~= al*silu(s*m + b) + c0        (|err| <= 0.019 on |z|<=6.5,
                                            half-normal-weighted bias ~ 0)
  Host re-encodes (byte-level only): z8 = fp8(c*p) via sign XOR, c8 = +-1.
  Device per tile [128, RB, 1000]:
      m8  = z8 | 0x80                      (DVE int32 bitwise, 4 B/elem packed)
      F8  = silu(s*m8 + b)                 (ACT, the single activation pass)
      p8  = (c8 & 0x80) ^ z8               (DVE scalar_tensor_tensor int32)
      cF8 = (c8 & 0x80) ^ F8               (DVE scalar_tensor_tensor int32)
      per-class sums of {z, p, F, cF, c} via fp8 matmuls with ONE-HOT
      [128,32] weights: quantity q lands in PSUM row 32*j + q where j is the
      column-group of the 4-way tile_position col tiling (4 row-blocks in
      flight concurrently on the PE array); 2 PSUM banks total.
  Host combine: Sa = z/2-sum + al*F-sum + c0*N, Sw = p/2-sum + al*cF-sum
  + c0*Sc, then the exact per-class division + valid-class mean (f64).

  numpy sim of the full quantized pipeline vs the f32 reference shows final
  rel err ~6e-4 (tolerance 2e-2). HBM traffic 2 B/elem (~23 us/core floor).
"""
import os
import numpy as np
import ml_dtypes

import concourse.bacc as bacc
import concourse.bass_utils as _bu
import concourse.tile as tile
from concourse import mybir
from concourse.bass_utils import run_bass_kernel_spmd

if os.environ.get("KERNEL_LDW_OPT", "0") == "1":
    # walrus's LDWEIGHTS optimizer dedups the identical per-matmul weight
    # reloads that otherwise break the 4-way col-tile concurrency.
    _orig_run_command = _bu.run_command

    def _patched_run_command(cmd, **kw):
        cmd = ["--enable-ldw-opt=true" if c == "--enable-ldw-opt=false" else c
               for c in cmd]
        return _orig_run_command(cmd, **kw)

    _bu.run_command = _patched_run_command

N = 32768
C = 1000
NCORES = 8
NSHARD = N // NCORES          # 4096 rows per core
P = 128                       # partitions
NBLK = NSHARD // P            # 32 row-blocks of 128 rows
HALF = C // 2                 # 500-col matmul halves (one PSUM bank each)
NT = 4                        # col-tile groups (concurrent matmuls)

# silu fit of E(m) = ln(2cosh(m/2)) on m in [-6.5, 0], half-normal weight
AL = 0.648334
FS = -0.699517
FB = -0.743431
C0 = 0.833047

SIGN32 = -2139062144          # 0x80808080 as signed int32


def _schedule():
    env = os.environ.get("KERNEL_SCHED")
    if env:
        sched = [int(x) for x in env.split(",")]
    else:
        # small first iters so the first ACTs start as soon as small DMAs
        # land; small last iter shortens the exposed tail chain.
        sched = [1, 3] + [4] * 6 + [3, 1]
    assert sum(sched) == NBLK
    return sched


_nc_cache = None
LAST_RESULTS = None           # BassKernelResults of the most recent run (for test harness)


def _build():
    fp8 = mybir.dt.float8e4
    i32 = mybir.dt.int32
    f32 = mybir.dt.float32
    Silu = mybir.ActivationFunctionType.Silu
    XOR = mybir.AluOpType.bitwise_xor
    AND = mybir.AluOpType.bitwise_and
    OR = mybir.AluOpType.bitwise_or

    bufs = int(os.environ.get("KERNEL_BUFS", "6"))

    nc = bacc.Bacc("TRN2", target_bir_lowering=False, debug=False, num_devices=NCORES)
    # z and c interleaved per row-block: [NBLK, 2, P, C] so each iteration
    # needs a single DMA (fewer queue issues + semaphores).
    zc_d = nc.dram_tensor("zc", [NBLK * 2 * P, C], fp8, kind="ExternalInput")
    # iteration-0's m tile comes straight from the host: the first ACT then
    # waits only on this small DMA, not on DMA + a DVE OR hop
    m0_d = nc.dram_tensor("m0", [P, C], fp8, kind="ExternalInput")
    sums = nc.dram_tensor("sums", [P, 2 * 512], f32, kind="ExternalOutput")
    # the last block's F/cF partial sums drain separately (small tail DMA)
    sums_t = nc.dram_tensor("sums_t", [32, 2 * 512], f32, kind="ExternalOutput")

    zcv = zc_d.ap().rearrange("(b t p) f -> p b t f", p=P, t=2)
    sched = _schedule()

    with tile.TileContext(nc) as tc:
        with (
            tc.tile_pool(name="work", bufs=bufs) as work,
            tc.tile_pool(name="singles", bufs=1) as singles,
            tc.tile_pool(name="psum", bufs=1, space="PSUM") as psum,
        ):
            bias = singles.tile([P, 1], f32)
            nc.vector.memset(bias, FB)
            msk = singles.tile([P, 1], i32)
            nc.vector.memset(msk, SIGN32)
            # one-hot [128, 32] fp8 weights, one per summed quantity
            whot = singles.tile([P, 5 * 32], fp8)
            nc.vector.memset(whot, 0.0)
            w3 = whot.rearrange("p (q f) -> p q f", q=5)
            for q in range(5):
                nc.vector.memset(w3[:, q, q:q + 1], 1.0)

            # Warm the silu table off the critical path: the table load
            # attaches to this data-independent activation and executes
            # during the preamble shadow (without it, the load lands right
            # before the first real ACT and delays the stream by ~1us).
            warm = singles.tile([1, 8], f32)
            nc.vector.memset(warm, 1.0)
            nc.scalar.activation(warm, warm, Silu)

            ps = [psum.tile([P, 512], f32, name=f"ps{h}") for h in range(2)]
            # small tail banks for the last block's F/cF (group 0 only)
            pst = [psum.tile([32, 512], f32, name=f"pst{h}") for h in range(2)]

            # block -> col-group mapping, rotated so the LAST block lands in
            # group 0 (partitions 0-31: a small tail tile can drain it)
            def grp(b):
                return (b + 1) % NT

            # start/stop bookkeeping per (tile j, half h) accumulation region
            started = [[False] * 2 for _ in range(NT)]
            n_mm = [[0] * 2 for _ in range(NT)]
            for b in range(NBLK):
                n_mm[grp(b)][0] += 5
                n_mm[grp(b)][1] += 5
            # the last block's sums all go to the tail banks instead, so the
            # main banks close (and drain) one iteration early
            n_mm[grp(NBLK - 1)][0] -= 5
            n_mm[grp(NBLK - 1)][1] -= 5
            seen = [[0] * 2 for _ in range(NT)]
            started_t = [False] * 2
            seen_t = [0] * 2

            def emit_mms(quants, s0, k, tail=False):
                for q, t3 in quants:
                    for h in range(2):
                        cs = slice(h * HALF, (h + 1) * HALF)
                        for bl in range(k):
                            j = grp(s0 + bl)
                            if tail:
                                assert j == 0 and k == 1
                                st = not started_t[h]
                                started_t[h] = True
                                seen_t[h] += 1
                                sp = seen_t[h] == 5
                                out = pst[h][0:32, 0:HALF]
                            else:
                                st = not started[j][h]
                                started[j][h] = True
                                seen[j][h] += 1
                                sp = seen[j][h] == n_mm[j][h]
                                out = ps[h][32 * j:32 * j + 32, 0:HALF]
                            nc.tensor.matmul(
                                out,
                                w3[:, q, :],
                                t3[:, bl, cs],
                                start=st, stop=sp,
                                tile_position=(0, 32 * j),
                            )

            def emit_cf(st, tail=False):
                # ACT-dependent work of iteration `st` — emitted one
                # iteration late so the DVE-queue op that waits on the ACT
                # never sits ahead of the next iteration's OR op.
                k, ci_i, ft_i, s0 = st
                # F matmuls first: they wait only on the ACT, and overlap
                # the cF XOR on the PE array
                f3 = ft_i.rearrange("p (b f) -> p b f", b=k)
                emit_mms(((2, f3),), s0, k, tail=tail)
                cft = work.tile([P, k * C], fp8, tag="cft")
                nc.vector.scalar_tensor_tensor(
                    cft.bitcast(i32).rearrange("p (b f) -> p b f", b=k),
                    ci_i,
                    msk,
                    ft_i.bitcast(i32).rearrange("p (b f) -> p b f", b=k),
                    AND, XOR,
                )
                cf3 = cft.rearrange("p (b f) -> p b f", b=k)
                emit_mms(((3, cf3),), s0, k, tail=tail)

            assert sched[0] == 1
            m0t = singles.tile([P, C], fp8)
            nc.sync.dma_start(out=m0t, in_=m0_d.ap())

            s = 0
            pend = None
            for i, k in enumerate(sched):
                zct = work.tile([P, k * 2 * C], fp8, tag="zct")
                zc4 = zct.rearrange("p (b t f) -> p b t f", b=k, t=2)
                nc.sync.dma_start(out=zc4, in_=zcv[:, s:s + k])
                z3 = zc4[:, :, 0, :]              # [P, k, C] stride 2C
                c3 = zc4[:, :, 1, :]
                zi = zc4.bitcast(i32)[:, :, 0, :]
                ci = zc4.bitcast(i32)[:, :, 1, :]

                mt = work.tile([P, k * C], fp8, tag="mt")
                ft = work.tile([P, k * C], fp8, tag="ft")
                pt = work.tile([P, k * C], fp8, tag="pt")
                if i == 0:
                    mt = m0t
                else:
                    nc.vector.tensor_scalar(
                        mt.bitcast(i32).rearrange("p (b f) -> p b f", b=k),
                        zi, msk, None, OR,
                    )
                nc.scalar.activation(ft, mt, Silu, bias=bias, scale=FS)
                # p XOR before the ACT-gated cF op: it is DMA-gated and
                # early-runnable, and this keeps it out of the exposed tail
                # chain between cF_{N-2} and cF_{N-1}
                nc.vector.scalar_tensor_tensor(
                    pt.bitcast(i32).rearrange("p (b f) -> p b f", b=k),
                    ci, msk, zi, AND, XOR,
                )
                if pend is not None:
                    emit_cf(pend)
                p3 = pt.rearrange("p (b f) -> p b f", b=k)
                emit_mms(((0, z3), (1, p3), (4, c3)), s, k,
                         tail=(i == len(sched) - 1))
                pend = (k, ci, ft, s)
                s += k

            # main banks are complete before the last silu even finishes:
            # drain + DMA them during the tail
            so = singles.tile([P, 2 * 512], f32)
            nc.scalar.copy(so[:, 0:512], ps[0])
            nc.vector.tensor_copy(so[:, 512:1024], ps[1])
            nc.sync.dma_start(out=sums.ap(), in_=so)

            # last iteration's F/cF into the small tail banks
            emit_cf(pend, tail=True)
            sot = singles.tile([32, 2 * 512], f32)
            nc.scalar.copy(sot[:, 0:512], pst[0])
            nc.vector.tensor_copy(sot[:, 512:1024], pst[1])
            nc.sync.dma_start(out=sums_t.ap(), in_=sot)

    nc.compile()
    return nc


def _encode_inputs(pred_y, true_y):
    """Byte-level re-encodings: z8 = fp8(c*p) via sign XOR, c8 = +-1 fp8.
    Returned interleaved per core as [NBLK, 2, P, C] so one DMA per
    iteration covers both tensors."""
    fp8 = ml_dtypes.float8_e4m3
    tb = true_y.astype(np.uint8)
    p8 = pred_y.astype(fp8)
    z8 = (p8.view(np.uint8) ^ (tb << 7)).view(fp8)
    c8 = (0x38 | (tb << 7)).view(fp8)  # +1.0 = 0x38, -1.0 = 0xB8
    zc = np.empty((NCORES, NBLK, 2, P, C), dtype=np.uint8)
    zc[:, :, 0] = z8.view(np.uint8).reshape(NCORES, NBLK, P, C)
    zc[:, :, 1] = c8.view(np.uint8).reshape(NCORES, NBLK, P, C)
    return zc.view(fp8)


def kernel(pred_y, true_y):
    global _nc_cache, LAST_RESULTS
    pred_y = np.asarray(pred_y, dtype=np.float32)
    true_y = np.asarray(true_y, dtype=np.int32)
    assert pred_y.shape == (N, C) and true_y.shape == (N, C)

    if _nc_cache is None:
        _nc_cache = _build()
    nc = _nc_cache

    zc = _encode_inputs(pred_y, true_y)
    in_maps = [
        {
            "zc": zc[k].reshape(NBLK * 2 * P, C),
            "m0": np.ascontiguousarray(
                (zc[k, 0, 0].view(np.uint8) | 0x80).view(zc.dtype)
            ),
        }
        for k in range(NCORES)
    ]

    trace = os.environ.get("KERNEL_TRACE") == "1"
    if trace:
        try:
            from antenv.axon_hooks import get_axon_ntff_profile_hook
            trace = get_axon_ntff_profile_hook() is not None
        except ImportError:
            trace = False
    res = run_bass_kernel_spmd(
        nc, in_maps, core_ids=list(range(NCORES)), trace=trace
    )
    LAST_RESULTS = res

    S = np.stack([r["sums"] for r in res.results]).astype(np.float64)  # [8, 128, 1024]
    St = np.stack([r["sums_t"] for r in res.results]).astype(np.float64)  # [8, 32, 1024]
    tot = S.sum(axis=0)
    tot_t = St.sum(axis=0)
    V = np.zeros((5, C))
    for q in range(5):
        for h in range(2):
            acc = np.zeros(HALF)
            for j in range(NT):
                acc += tot[32 * j + q, h * 512:h * 512 + HALF]
            # the last block's sums live in the tail banks
            acc += tot_t[q, h * 512:h * 512 + HALF]
            V[q, h * HALF:(h + 1) * HALF] = acc
    Sz, Sp, SF, ScF, Sc = V

    Sa = 0.5 * Sz + AL * SF + C0 * N
    Sw = 0.5 * Sp + AL * ScF + C0 * Sc
    B = (Sa - Sw) / 2.0
    Cn = (Sa + Sw) / 2.0
    n_pos = (N - Sc) / 2.0
    n_neg = (N + Sc) / 2.0
    valid = (n_pos > 0) & (n_neg > 0)
    loss_c = B / np.maximum(n_pos, 1.0) + Cn / np.maximum(n_neg, 1.0)
    n_valid = max(float(valid.sum()), 1.0)
    out = np.where(valid, loss_c, 0.0).sum() / n_valid
    return np.float32(out)


# revision 83
# speedup vs baseline: 1.0432x; 1.0432x over previous
"""TRN2 Bass kernel for nn_NaiveReweightedLoss (reweighted per-class BCE-style loss).

Reference semantics (N=32768 samples, C=1000 classes, t in {0,1}):
    B_c = sum_i t * softplus(-p),  C_c = sum_i (1-t) * softplus(p)
    n_pos_c = sum_i t, n_neg_c = N - n_pos_c
    valid = (n_pos>0)&(n_neg>0)
    loss = mean over valid classes of B/max(n_pos,1) + C/max(n_neg,1)

Device algorithm (data-parallel over rows, 8 cores x 4096 rows):
  Even/odd split of softplus kills one of the two ACT passes the exact
  exp+ln route needs:  softplus(z) = z/2 + E(m),  m = -|z|,
  E(m) = ln(2 cosh(m/2)) is EVEN, approximated by one tabled function:
      E(m) ~= al*silu(s*m + b) + c0        (|err| <= 0.019 on |z|<=6.5,
                                            half-normal-weighted bias ~ 0)
  Host re-encodes (byte-level only): z8 = fp8(c*p) via sign XOR, c8 = +-1.
  Device per tile [128, RB, 1000]:
      m8  = z8 | 0x80                      (DVE int32 bitwise, 4 B/elem packed)
      F8  = silu(s*m8 + b)                 (ACT, the single activation pass)
      p8  = (c8 & 0x80) ^ z8               (DVE scalar_tensor_tensor int32)
      cF8 = (c8 & 0x80) ^ F8               (DVE scalar_tensor_tensor int32)
      per-class sums of {z, p, F, cF, c} via fp8 matmuls with ONE-HOT
      [128,32] weights: quantity q lands in PSUM row 32*j + q where j is the
      column-group of the 4-way tile_position col tiling (4 row-blocks in
      flight concurrently on the PE array); 2 PSUM banks total.
  Host combine: Sa = z/2-sum + al*F-sum + c0*N, Sw = p/2-sum + al*cF-sum
  + c0*Sc, then the exact per-class division + valid-class mean (f64).

  numpy sim of the full quantized pipeline vs the f32 reference shows final
  rel err ~6e-4 (tolerance 2e-2). HBM traffic 2 B/elem (~23 us/core floor).
"""
import os
import numpy as np
import ml_dtypes

import concourse.bacc as bacc
import concourse.bass_utils as _bu
import concourse.tile as tile
from concourse import mybir
from concourse.bass_utils import run_bass_kernel_spmd

if os.environ.get("KERNEL_LDW_OPT", "0") == "1":
    # walrus's LDWEIGHTS optimizer dedups the identical per-matmul weight
    # reloads that otherwise break the 4-way col-tile concurrency.
    _orig_run_command = _bu.run_command

    def _patched_run_command(cmd, **kw):
        cmd = ["--enable-ldw-opt=true" if c == "--enable-ldw-opt=false" else c
               for c in cmd]
        return _orig_run_command(cmd, **kw)

    _bu.run_command = _patched_run_command

N = 32768
C = 1000
NCORES = 8
NSHARD = N // NCORES          # 4096 rows per core
P = 128                       # partitions
NBLK = NSHARD // P            # 32 row-blocks of 128 rows
HALF = C // 2                 # 500-col matmul halves (one PSUM bank each)
NT = 4                        # col-tile groups (concurrent matmuls)

# silu fit of E(m) = ln(2cosh(m/2)) on m in [-6.5, 0], half-normal weight
AL = 0.648334
FS = -0.699517
FB = -0.743431
C0 = 0.833047

SIGN32 = -2139062144          # 0x80808080 as signed int32


def _schedule():
    env = os.environ.get("KERNEL_SCHED")
    if env:
        sched = [int(x) for x in env.split(",")]
    else:
        # small first iters so the first ACTs start as soon as small DMAs
        # land; small last iter shortens the exposed tail chain.
        sched = [1, 3] + [4] * 6 + [3, 1]
    assert sum(sched) == NBLK
    return sched


_nc_cache = None
LAST_RESULTS = None           # BassKernelResults of the most recent run (for test harness)


def _build():
    fp8 = mybir.dt.float8e4
    i32 = mybir.dt.int32
    f32 = mybir.dt.float32
    Silu = mybir.ActivationFunctionType.Silu
    XOR = mybir.AluOpType.bitwise_xor
    AND = mybir.AluOpType.bitwise_and
    OR = mybir.AluOpType.bitwise_or

    bufs = int(os.environ.get("KERNEL_BUFS", "6"))

    nc = bacc.Bacc("TRN2", target_bir_lowering=False, debug=False, num_devices=NCORES)
    # z and c interleaved per row-block: [NBLK, 2, P, C] so each iteration
    # needs a single DMA (fewer queue issues + semaphores).
    zc_d = nc.dram_tensor("zc", [NBLK * 2 * P, C], fp8, kind="ExternalInput")
    # iteration-0's m tile comes straight from the host: the first ACT then
    # waits only on this small DMA, not on DMA + a DVE OR hop
    m0_d = nc.dram_tensor("m0", [P, C], fp8, kind="ExternalInput")
    sums = nc.dram_tensor("sums", [P, 2 * 512], f32, kind="ExternalOutput")
    # the last block's F/cF partial sums drain separately (small tail DMA)
    sums_t = nc.dram_tensor("sums_t", [32, 2 * 512], f32, kind="ExternalOutput")

    zcv = zc_d.ap().rearrange("(b t p) f -> p b t f", p=P, t=2)
    sched = _schedule()

    with tile.TileContext(nc) as tc:
        with (
            tc.tile_pool(name="work", bufs=bufs) as work,
            tc.tile_pool(name="singles", bufs=1) as singles,
            tc.tile_pool(name="psum", bufs=1, space="PSUM") as psum,
        ):
            bias = singles.tile([P, 1], f32)
            nc.vector.memset(bias, FB)
            msk = singles.tile([P, 1], i32)
            nc.vector.memset(msk, SIGN32)
            # one-hot [128, 32] fp8 weights, one per summed quantity
            whot = singles.tile([P, 5 * 32], fp8)
            nc.vector.memset(whot, 0.0)
            w3 = whot.rearrange("p (q f) -> p q f", q=5)
            for q in range(5):
                nc.vector.memset(w3[:, q, q:q + 1], 1.0)

            # Warm the silu table off the critical path: the table load
            # attaches to this data-independent activation and executes
            # during the preamble shadow (without it, the load lands right
            # before the first real ACT and delays the stream by ~1us).
            warm = singles.tile([1, 8], f32)
            nc.vector.memset(warm, 1.0)
            nc.scalar.activation(warm, warm, Silu)

            ps = [psum.tile([P, 512], f32, name=f"ps{h}") for h in range(2)]
            # small tail banks for the last block's F/cF (group 0 only)
            pst = [psum.tile([32, 512], f32, name=f"pst{h}") for h in range(2)]

            # block -> col-group mapping, rotated so the LAST block lands in
            # group 0 (partitions 0-31: a small tail tile can drain it)
            def grp(b):
                return (b + 1) % NT

            # start/stop bookkeeping per (tile j, half h) accumulation region
            started = [[False] * 2 for _ in range(NT)]
            n_mm = [[0] * 2 for _ in range(NT)]
            for b in range(NBLK):
                n_mm[grp(b)][0] += 5
                n_mm[grp(b)][1] += 5
            # the last block's sums all go to the tail banks instead, so the
            # main banks close (and drain) one iteration early
            n_mm[grp(NBLK - 1)][0] -= 5
            n_mm[grp(NBLK - 1)][1] -= 5
            seen = [[0] * 2 for _ in range(NT)]
            started_t = [False] * 2
            seen_t = [0] * 2

            def emit_mms(quants, s0, k, tail=False):
                for q, t3 in quants:
                    for h in range(2):
                        cs = slice(h * HALF, (h + 1) * HALF)
                        for bl in range(k):
                            j = grp(s0 + bl)
                            if tail:
                                assert j == 0 and k == 1
                                st = not started_t[h]
                                started_t[h] = True
                                seen_t[h] += 1
                                sp = seen_t[h] == 5
                                out = pst[h][0:32, 0:HALF]
                            else:
                                st = not started[j][h]
                                started[j][h] = True
                                seen[j][h] += 1
                                sp = seen[j][h] == n_mm[j][h]
                                out = ps[h][32 * j:32 * j + 32, 0:HALF]
                            nc.tensor.matmul(
                                out,
                                w3[:, q, :],
                                t3[:, bl, cs],
                                start=st, stop=sp,
                                tile_position=(0, 32 * j),
                            )

            def emit_cf(st, tail=False):
                # ACT-dependent work of iteration `st` — emitted one
                # iteration late so the DVE-queue op that waits on the ACT
                # never sits ahead of the next iteration's OR op.
                k, ci_i, ft_i, s0 = st
                # F matmuls first: they wait only on the ACT, and overlap
                # the cF XOR on the PE array
                f3 = ft_i.rearrange("p (b f) -> p b f", b=k)
                emit_mms(((2, f3),), s0, k, tail=tail)
                cft = work.tile([P, k * C], fp8, tag="cft")
                nc.vector.scalar_tensor_tensor(
                    cft.bitcast(i32).rearrange("p (b f) -> p b f", b=k),
                    ci_i,
                    msk,
                    ft_i.bitcast(i32).rearrange("p (b f) -> p b f", b=k),
                    AND, XOR,
                )
                cf3 = cft.rearrange("p (b f) -> p b f", b=k)
                emit_mms(((3, cf3),), s0, k, tail=tail)

            assert sched[0] == 1
            m0t = singles.tile([P, C], fp8)
            nc.sync.dma_start(out=m0t, in_=m0_d.ap())

            s = 0
            pend = None
            for i, k in enumerate(sched):
                zct = work.tile([P, k * 2 * C], fp8, tag="zct")
                zc4 = zct.rearrange("p (b t f) -> p b t f", b=k, t=2)
                nc.sync.dma_start(out=zc4, in_=zcv[:, s:s + k])
                z3 = zc4[:, :, 0, :]              # [P, k, C] stride 2C
                c3 = zc4[:, :, 1, :]
                zi = zc4.bitcast(i32)[:, :, 0, :]
                ci = zc4.bitcast(i32)[:, :, 1, :]

                mt = work.tile([P, k * C], fp8, tag="mt")
                ft = work.tile([P, k * C], fp8, tag="ft")
                pt = work.tile([P, k * C], fp8, tag="pt")
                if i == 0:
                    mt = m0t
                else:
                    nc.vector.tensor_scalar(
                        mt.bitcast(i32).rearrange("p (b f) -> p b f", b=k),
                        zi, msk, None, OR,
                    )
                nc.scalar.activation(ft, mt, Silu, bias=bias, scale=FS)
                if pend is not None:
                    emit_cf(pend)
                nc.vector.scalar_tensor_tensor(
                    pt.bitcast(i32).rearrange("p (b f) -> p b f", b=k),
                    ci, msk, zi, AND, XOR,
                )
                p3 = pt.rearrange("p (b f) -> p b f", b=k)
                emit_mms(((0, z3), (1, p3), (4, c3)), s, k,
                         tail=(i == len(sched) - 1))
                pend = (k, ci, ft, s)
                s += k

            # main banks are complete before the last silu even finishes:
            # drain + DMA them during the tail
            so = singles.tile([P, 2 * 512], f32)
            nc.scalar.copy(so[:, 0:512], ps[0])
            nc.vector.tensor_copy(so[:, 512:1024], ps[1])
            nc.sync.dma_start(out=sums.ap(), in_=so)

            # last iteration's F/cF into the small tail banks
            emit_cf(pend, tail=True)
            sot = singles.tile([32, 2 * 512], f32)
            nc.scalar.copy(sot[:, 0:512], pst[0])
            nc.vector.tensor_copy(sot[:, 512:1024], pst[1])
            nc.sync.dma_start(out=sums_t.ap(), in_=sot)

    nc.compile()
    return nc


def _encode_inputs(pred_y, true_y):
    """Byte-level re-encodings: z8 = fp8(c*p) via sign XOR, c8 = +-1 fp8.
    Returned interleaved per core as [NBLK, 2, P, C] so one DMA per
    iteration covers both tensors."""
    fp8 = ml_dtypes.float8_e4m3
    tb = true_y.astype(np.uint8)
    p8 = pred_y.astype(fp8)
    z8 = (p8.view(np.uint8) ^ (tb << 7)).view(fp8)
    c8 = (0x38 | (tb << 7)).view(fp8)  # +1.0 = 0x38, -1.0 = 0xB8
    zc = np.empty((NCORES, NBLK, 2, P, C), dtype=np.uint8)
    zc[:, :, 0] = z8.view(np.uint8).reshape(NCORES, NBLK, P, C)
    zc[:, :, 1] = c8.view(np.uint8).reshape(NCORES, NBLK, P, C)
    return zc.view(fp8)


def kernel(pred_y, true_y):
    global _nc_cache, LAST_RESULTS
    pred_y = np.asarray(pred_y, dtype=np.float32)
    true_y = np.asarray(true_y, dtype=np.int32)
    assert pred_y.shape == (N, C) and true_y.shape == (N, C)

    if _nc_cache is None:
        _nc_cache = _build()
    nc = _nc_cache

    zc = _encode_inputs(pred_y, true_y)
    in_maps = [
        {
            "zc": zc[k].reshape(NBLK * 2 * P, C),
            "m0": np.ascontiguousarray(
                (zc[k, 0, 0].view(np.uint8) | 0x80).view(zc.dtype)
            ),
        }
        for k in range(NCORES)
    ]

    trace = os.environ.get("KERNEL_TRACE") == "1"
    if trace:
        try:
            from antenv.axon_hooks import get_axon_ntff_profile_hook
            trace = get_axon_ntff_profile_hook() is not None
        except ImportError:
            trace = False
    res = run_bass_kernel_spmd(
        nc, in_maps, core_ids=list(range(NCORES)), trace=trace
    )
    LAST_RESULTS = res

    S = np.stack([r["sums"] for r in res.results]).astype(np.float64)  # [8, 128, 1024]
    St = np.stack([r["sums_t"] for r in res.results]).astype(np.float64)  # [8, 32, 1024]
    tot = S.sum(axis=0)
    tot_t = St.sum(axis=0)
    V = np.zeros((5, C))
    for q in range(5):
        for h in range(2):
            acc = np.zeros(HALF)
            for j in range(NT):
                acc += tot[32 * j + q, h * 512:h * 512 + HALF]
            # the last block's sums live in the tail banks
            acc += tot_t[q, h * 512:h * 512 + HALF]
            V[q, h * HALF:(h + 1) * HALF] = acc
    Sz, Sp, SF, ScF, Sc = V

    Sa = 0.5 * Sz + AL * SF + C0 * N
    Sw = 0.5 * Sp + AL * ScF + C0 * Sc
    B = (Sa - Sw) / 2.0
    Cn = (Sa + Sw) / 2.0
    n_pos = (N - Sc) / 2.0
    n_neg = (N + Sc) / 2.0
    valid = (n_pos > 0) & (n_neg > 0)
    loss_c = B / np.maximum(n_pos, 1.0) + Cn / np.maximum(n_neg, 1.0)
    n_valid = max(float(valid.sum()), 1.0)
    out = np.where(valid, loss_c, 0.0).sum() / n_valid
    return np.float32(out)
